# revision 5
# baseline (speedup 1.0000x reference)
"""Trainium2 Bass kernel for nn_NodeEncodeInterface (GNN message passing).

Strategy (per sharding hint: shard nodes/edges with graph-partitioned edge
cuts, replicate small embeddings + MLP weights):
 - Host: partitions edges by owner core (src chunk), filters carbon->hydrogen
   edges, greedily packs them into static 128-edge columns grouped by target
   carbon rank, so the device kernel is fully static (no scatter, no RMW).
 - Device (8 NeuronCores, SPMD): gathers x rows for message sources and
   carbon nodes, computes the segment-mean via selection-matrix matmuls in
   PSUM, then runs both Projection MLPs (fp32 TensorEngine) in transposed
   orientation, emitting compact per-carbon outputs.
 - Host: scatters compact outputs into the full [N, 2] result.
"""

import numpy as np

import concourse.bass as bass
import concourse.mybir as mybir
import concourse.tile as tile_mod
from concourse.tile import TileContext
from concourse.masks import make_identity
from concourse.vector_clock import ScopedClock
from concourse import bass_utils

f32 = mybir.dt.float32
i32 = mybir.dt.int32
ALU = mybir.AluOpType

N = 300000
HID = 256
EMB = 32
NSOLV = 9
NCORES = 8
CH = N // NCORES          # 37500 nodes per core

NCOL = 32                 # static 128-edge columns per core
RPC = 64                  # carbon-rank slots per column
SLOTS = NCOL * RPC        # 2048 output slots per core
VE = NCOL * 128           # 4096 edge slots per core
GRP = 512                 # MLP rank-group width
NGRP = SLOTS // GRP       # 4 groups
FH = EMB + HID            # 288 (mlp input dim)


# ---------------------------------------------------------------------------
# walrus workaround: this build rejects >1 semaphore wait on several lowered
# instruction encodings; split extra waits onto same-engine NoOps.
# ---------------------------------------------------------------------------
def _patched_drain_and_barrier(self, tick_clock, wait_clock):
    nc = self.nc
    drain_inst = nc.sync.drain()
    wait_clock.add_sem_waits(
        drain_inst.ins, ScopedClock({None: tick_clock.global_clock})
    )
    si = drain_inst.ins.sync_info
    waits = list(si.on_wait)
    if len(waits) > 1:
        si.on_wait = waits[:1]
        for w in waits[1:]:
            extra = nc.sync.drain()
            extra.ins.sync_info = mybir.SyncInfo(on_wait=[w], on_update=[])
    nc.all_engine_barrier()
    popped = nc._tile_sem_poison_stack.pop()
    assert popped is self._sem_poison
    nc.clear_and_free_semaphores(list(self.sems.allocated().values()))
    nc.all_engine_barrier()


tile_mod.TileContext._drain_and_barrier = _patched_drain_and_barrier


def _split_waits(nc, maxw=1):
    fn = nc.m.functions[0]
    for bb in fn.blocks:
        out = []
        changed = False
        for inst in bb.instructions:
            si = inst.sync_info
            waits = list(si.on_wait) if si is not None else []
            if len(waits) > maxw:
                changed = True
                for i in range(0, len(waits) - maxw, maxw):
                    nop = mybir.InstNoOp(
                        name=nc.get_next_instruction_name(),
                        text_hint="waitsplit",
                        bass_nofuse=True,
                    )
                    nop.engine = inst.engine
                    nop.sync_info = mybir.SyncInfo(
                        on_wait=waits[i : i + maxw], on_update=[]
                    )
                    out.append(nop)
                si.on_wait = waits[len(waits) - maxw :]
            out.append(inst)
        if changed:
            bb.instructions[:] = out
    return nc


# ---------------------------------------------------------------------------
# device kernel
# ---------------------------------------------------------------------------
def _build():
    nc = bass.Bass("TRN2")
    x = nc.dram_tensor("x", [N, HID], f32, kind="ExternalInput")
    c_emb = nc.dram_tensor("c_emb", [NSOLV, EMB], f32, kind="ExternalInput")
    h_emb = nc.dram_tensor("h_emb", [NSOLV, EMB], f32, kind="ExternalInput")
    cW1 = nc.dram_tensor("cW1", [FH, 256], f32, kind="ExternalInput")
    cb1 = nc.dram_tensor("cb1", [256], f32, kind="ExternalInput")
    cW2 = nc.dram_tensor("cW2", [256, 512], f32, kind="ExternalInput")
    cb2 = nc.dram_tensor("cb2", [512], f32, kind="ExternalInput")
    cW3 = nc.dram_tensor("cW3", [512, 1], f32, kind="ExternalInput")
    cb3 = nc.dram_tensor("cb3", [1], f32, kind="ExternalInput")
    hW1 = nc.dram_tensor("hW1", [FH, 256], f32, kind="ExternalInput")
    hb1 = nc.dram_tensor("hb1", [256], f32, kind="ExternalInput")
    hW2 = nc.dram_tensor("hW2", [256, 512], f32, kind="ExternalInput")
    hb2 = nc.dram_tensor("hb2", [512], f32, kind="ExternalInput")
    hW3 = nc.dram_tensor("hW3", [512, 1], f32, kind="ExternalInput")
    hb3 = nc.dram_tensor("hb3", [1], f32, kind="ExternalInput")
    # per-core packed edge/carbon structure (host prepared)
    vdst = nc.dram_tensor("vdst", [128, NCOL], i32, kind="ExternalInput")
    vsol = nc.dram_tensor("vsol", [128, NCOL], i32, kind="ExternalInput")
    vloc = nc.dram_tensor("vloc", [128, NCOL], i32, kind="ExternalInput")
    vw = nc.dram_tensor("vw", [128, NCOL], f32, kind="ExternalInput")
    cxid = nc.dram_tensor("cxid", [128, SLOTS // 128], i32, kind="ExternalInput")
    csol = nc.dram_tensor("csol", [128, SLOTS // 128], i32, kind="ExternalInput")
    invr = nc.dram_tensor("invr", [128, SLOTS], f32, kind="ExternalInput")
    out = nc.dram_tensor("out", [2, SLOTS], f32, kind="ExternalOutput")

    CCOL = SLOTS // 128  # 16 carbon-gather columns

    with TileContext(nc) as tc:
        with (
            tc.tile_pool(name="const", bufs=1) as cst,
            tc.tile_pool(name="wts", bufs=1) as wts,
            tc.tile_pool(name="edge", bufs=1) as edg,
            tc.tile_pool(name="work", bufs=3) as wrk,
            tc.tile_pool(name="hsum", bufs=1) as hsp,
            tc.tile_pool(name="mlp", bufs=1) as mlp,
            tc.tile_pool(name="pse", bufs=1, space="PSUM") as pse,
            tc.tile_pool(name="psS", bufs=1, space="PSUM") as psS,
            tc.tile_pool(name="psL", bufs=2, space="PSUM") as psL,
            tc.tile_pool(name="outp", bufs=1) as outp,
        ):
            ident = cst.tile([128, 128], f32)
            make_identity(nc, ident[:])
            iota9 = cst.tile([128, NSOLV], i32)
            nc.gpsimd.iota(iota9[:], pattern=[[1, NSOLV]], base=0, channel_multiplier=0)
            iota9f = cst.tile([128, NSOLV], f32)
            nc.vector.tensor_copy(iota9f[:], iota9[:])
            iota64 = cst.tile([128, RPC], i32)
            nc.gpsimd.iota(iota64[:], pattern=[[1, RPC]], base=0, channel_multiplier=0)
            iota64f = cst.tile([128, RPC], f32)
            nc.vector.tensor_copy(iota64f[:], iota64[:])
            iotaP9 = cst.tile([NSOLV, 128], i32)
            nc.gpsimd.iota(iotaP9[:], pattern=[[0, 128]], base=0, channel_multiplier=1)
            iotaP9f = cst.tile([NSOLV, 128], f32)
            nc.vector.tensor_copy(iotaP9f[:], iotaP9[:])

            # ---- weights to SBUF ----
            w1h_a = wts.tile([128, 256], f32)   # hW1 x-rows 0..127   (= hW1[32:160])
            w1h_b = wts.tile([128, 256], f32)   # hW1 x-rows 128..255 (= hW1[160:288])
            nc.sync.dma_start(out=w1h_a[:], in_=hW1[EMB : EMB + 128, :])
            nc.sync.dma_start(out=w1h_b[:], in_=hW1[EMB + 128 : EMB + 256, :])
            w1c_a = wts.tile([128, 256], f32)
            w1c_b = wts.tile([128, 256], f32)
            nc.sync.dma_start(out=w1c_a[:], in_=cW1[EMB : EMB + 128, :])
            nc.sync.dma_start(out=w1c_b[:], in_=cW1[EMB + 128 : EMB + 256, :])
            w1h_e = wts.tile([EMB, 256], f32)   # hW1 emb-rows
            w1c_e = wts.tile([EMB, 256], f32)
            nc.sync.dma_start(out=w1h_e[:], in_=hW1[0:EMB, :])
            nc.sync.dma_start(out=w1c_e[:], in_=cW1[0:EMB, :])
            w2h = wts.tile([128, 2 * 512], f32)  # [k-chunk, chunk*512]
            w2c = wts.tile([128, 2 * 512], f32)
            for kc in range(2):
                nc.sync.dma_start(
                    out=w2h[:, kc * 512 : (kc + 1) * 512],
                    in_=hW2[kc * 128 : (kc + 1) * 128, :],
                )
                nc.sync.dma_start(
                    out=w2c[:, kc * 512 : (kc + 1) * 512],
                    in_=cW2[kc * 128 : (kc + 1) * 128, :],
                )
            w3h = wts.tile([128, 4], f32)       # hW3 chunks as columns
            w3c = wts.tile([128, 4], f32)
            nc.sync.dma_start(out=w3h[:], in_=hW3[:, 0].rearrange("(c p) -> p c", p=128))
            nc.sync.dma_start(out=w3c[:], in_=cW3[:, 0].rearrange("(c p) -> p c", p=128))
            b1h = wts.tile([128, 2], f32)       # hb1 transposed blocks
            b1c = wts.tile([128, 2], f32)
            nc.sync.dma_start(out=b1h[:], in_=hb1[:].rearrange("(c p) -> p c", p=128))
            nc.sync.dma_start(out=b1c[:], in_=cb1[:].rearrange("(c p) -> p c", p=128))
            b2h = wts.tile([128, 4], f32)
            b2c = wts.tile([128, 4], f32)
            nc.sync.dma_start(out=b2h[:], in_=hb2[:].rearrange("(c p) -> p c", p=128))
            nc.sync.dma_start(out=b2c[:], in_=cb2[:].rearrange("(c p) -> p c", p=128))
            b3h = wts.tile([1, 1], f32)
            b3c = wts.tile([1, 1], f32)
            nc.sync.dma_start(out=b3h[:], in_=hb3[None, :])
            nc.sync.dma_start(out=b3c[:], in_=cb3[None, :])

            # emb tables through W1: hU9 = h_emb @ hW1[:32]  ->  [9, 256]
            embT_ps = pse.tile([EMB, NSOLV], f32, tag="e")
            hembT = wts.tile([EMB, NSOLV], f32)
            cembT = wts.tile([EMB, NSOLV], f32)
            hembS = wrk.tile([NSOLV, EMB], f32)
            cembS = wrk.tile([NSOLV, EMB], f32)
            nc.sync.dma_start(out=hembS[:], in_=h_emb[:])
            nc.sync.dma_start(out=cembS[:], in_=c_emb[:])
            nc.tensor.transpose(embT_ps[:], hembS[:], ident[0:NSOLV, 0:NSOLV])
            nc.vector.tensor_copy(hembT[:], embT_ps[:])
            embT_ps2 = pse.tile([EMB, NSOLV], f32, tag="e")
            nc.tensor.transpose(embT_ps2[:], cembS[:], ident[0:NSOLV, 0:NSOLV])
            nc.vector.tensor_copy(cembT[:], embT_ps2[:])
            hU9_ps = pse.tile([NSOLV, 256], f32, tag="e")
            nc.tensor.matmul(hU9_ps[:], lhsT=hembT[:], rhs=w1h_e[:], start=True, stop=True)
            hU9 = wts.tile([NSOLV, 256], f32)
            nc.vector.tensor_copy(hU9[:], hU9_ps[:])
            cU9_ps = pse.tile([NSOLV, 256], f32, tag="e")
            nc.tensor.matmul(cU9_ps[:], lhsT=cembT[:], rhs=w1c_e[:], start=True, stop=True)
            cU9 = wts.tile([NSOLV, 256], f32)
            nc.vector.tensor_copy(cU9[:], cU9_ps[:])

            # ---- edge structure ----
            vdstT = edg.tile([128, NCOL], i32)
            vsolT = edg.tile([128, NCOL], f32)
            vlocT = edg.tile([128, NCOL], f32)
            vwT = edg.tile([128, NCOL], f32)
            nc.sync.dma_start(out=vdstT[:], in_=vdst[:])
            vsol_i = edg.tile([128, NCOL], i32)
            nc.sync.dma_start(out=vsol_i[:], in_=vsol[:])
            nc.vector.tensor_copy(vsolT[:], vsol_i[:])
            vloc_i = edg.tile([128, NCOL], i32)
            nc.sync.dma_start(out=vloc_i[:], in_=vloc[:])
            nc.vector.tensor_copy(vlocT[:], vloc_i[:])
            nc.sync.dma_start(out=vwT[:], in_=vw[:])

            # H9 for all edges: [128, NCOL*9]
            H9 = edg.tile([128, NCOL * NSOLV], f32)
            nc.vector.tensor_tensor(
                out=H9[:].rearrange("p (k s) -> p k s", s=NSOLV),
                in0=vsolT[:].rearrange("p (k one) -> p k one", one=1).to_broadcast(
                    [128, NCOL, NSOLV]
                ),
                in1=iota9f[:].rearrange("p (k s) -> p k s", k=1).to_broadcast(
                    [128, NCOL, NSOLV]
                ),
                op=ALU.is_equal,
            )

            # x gather for edges: [128, NCOL*256]
            xg = edg.tile([128, NCOL * HID], f32)
            for i in range(NCOL):
                nc.gpsimd.indirect_dma_start(
                    out=xg[:, i * HID : (i + 1) * HID],
                    out_offset=None,
                    in_=x[:],
                    in_offset=bass.IndirectOffsetOnAxis(ap=vdstT[:, i : i + 1], axis=0),
                )

            # ---- segment sum via selection matmuls ----
            # h_sum^T tiles: hsA [128, SLOTS] (x dims 0-127), hsB (x 128-255),
            # hs9 [9, SLOTS] (solvent counts)
            hsA = hsp.tile([128, SLOTS], f32)
            hsB = hsp.tile([128, SLOTS], f32)
            hs9 = hsp.tile([NSOLV, SLOTS], f32)
            invT = hsp.tile([128, SLOTS], f32)
            nc.sync.dma_start(out=invT[:], in_=invr[:])

            for i in range(NCOL):
                # S[e, r] = w_e * (vloc_e == r)   [128, 64]
                S = wrk.tile([128, RPC], f32, tag="S")
                nc.vector.tensor_tensor(
                    out=S[:],
                    in0=vlocT[:, i : i + 1].to_broadcast([128, RPC]),
                    in1=iota64f[0:128, :],
                    op=ALU.is_equal,
                )
                nc.vector.tensor_scalar(
                    out=S[:], in0=S[:], scalar1=vwT[:, i : i + 1], scalar2=None,
                    op0=ALU.mult,
                )
                sl = slice(i * RPC, (i + 1) * RPC)
                pA = psS.tile([128, RPC], f32, tag="pA")
                pB = psS.tile([128, RPC], f32, tag="pB")
                p9 = psS.tile([NSOLV, RPC], f32, tag="p9")
                nc.tensor.matmul(pA[:], lhsT=xg[:, i * HID : i * HID + 128], rhs=S[:], start=True, stop=True)
                nc.tensor.matmul(pB[:], lhsT=xg[:, i * HID + 128 : (i + 1) * HID], rhs=S[:], start=True, stop=True)
                nc.tensor.matmul(p9[:], lhsT=H9[:, i * NSOLV : (i + 1) * NSOLV], rhs=S[:], start=True, stop=True)
                # average while copying out of PSUM
                nc.vector.tensor_tensor(out=hsA[:, sl], in0=pA[:], in1=invT[:, sl], op=ALU.mult)
                nc.vector.tensor_tensor(out=hsB[:, sl], in0=pB[:], in1=invT[:, sl], op=ALU.mult)
                nc.vector.tensor_tensor(out=hs9[:, sl], in0=p9[:], in1=invT[0:NSOLV, sl], op=ALU.mult)

            # ---- carbon-side inputs ----
            cxidT = edg.tile([128, CCOL], i32)
            nc.sync.dma_start(out=cxidT[:], in_=cxid[:])
            csol_i = edg.tile([128, CCOL], i32)
            nc.sync.dma_start(out=csol_i[:], in_=csol[:])
            csolF = edg.tile([128, CCOL], f32)
            nc.vector.tensor_copy(csolF[:], csol_i[:])

            xc = edg.tile([128, CCOL * HID], f32)
            for u in range(CCOL):
                nc.gpsimd.indirect_dma_start(
                    out=xc[:, u * HID : (u + 1) * HID],
                    out_offset=None,
                    in_=x[:],
                    in_offset=bass.IndirectOffsetOnAxis(ap=cxidT[:, u : u + 1], axis=0),
                )

            # transposed carbon x: xcT chunks [128, SLOTS] x 2
            xcTa = hsp.tile([128, SLOTS], f32)
            xcTb = hsp.tile([128, SLOTS], f32)
            for u in range(CCOL):
                for c, dstt in ((0, xcTa), (1, xcTb)):
                    tp = pse.tile([128, 128], f32, tag="e")
                    nc.tensor.transpose(
                        tp[:], xc[:, u * HID + c * 128 : u * HID + (c + 1) * 128], ident[:]
                    )
                    nc.vector.tensor_copy(dstt[:, u * 128 : (u + 1) * 128], tp[:])
            # carbon solvent one-hot transposed: H9c [9, SLOTS]
            H9c = hsp.tile([NSOLV, SLOTS], f32)
            for u in range(CCOL):
                srep_ps = pse.tile([128, 128], f32, tag="e")
                nc.tensor.transpose(
                    srep_ps[:], csolF[:, u : u + 1].to_broadcast([128, 128]), ident[:]
                )
                srep = wrk.tile([NSOLV, 128], f32, tag="srep_s")
                nc.vector.tensor_copy(srep[:], srep_ps[0:NSOLV, :])
                nc.vector.tensor_tensor(
                    out=H9c[:, u * 128 : (u + 1) * 128],
                    in0=iotaP9f[:],
                    in1=srep[:],
                    op=ALU.is_equal,
                )

            # ---- MLPs per rank group ----
            o2c = outp.tile([1, SLOTS], f32)
            o2h = outp.tile([1, SLOTS], f32)
            for g in range(NGRP):
                gs = slice(g * GRP, (g + 1) * GRP)
                # h-side L1: h1T [256, GRP] in 2 psum blocks
                h1s = mlp.tile([128, 2 * GRP], f32, tag="h1s")
                c1s = mlp.tile([128, 2 * GRP], f32, tag="c1s")
                for fb in range(2):
                    fsl = slice(fb * 128, (fb + 1) * 128)
                    ph = psL.tile([128, GRP], f32, tag="pl1")
                    nc.tensor.matmul(ph[:], lhsT=w1h_a[:, fsl], rhs=hsA[:, gs], start=True, stop=False)
                    nc.tensor.matmul(ph[:], lhsT=w1h_b[:, fsl], rhs=hsB[:, gs], start=False, stop=False)
                    nc.tensor.matmul(ph[:], lhsT=hU9[:, fsl], rhs=hs9[:, gs], start=False, stop=True)
                    nc.vector.tensor_scalar(
                        out=h1s[:, fb * GRP : (fb + 1) * GRP], in0=ph[:],
                        scalar1=b1h[:, fb : fb + 1], scalar2=None, op0=ALU.add,
                    )
                    pc = psL.tile([128, GRP], f32, tag="pl1")
                    nc.tensor.matmul(pc[:], lhsT=w1c_a[:, fsl], rhs=xcTa[:, gs], start=True, stop=False)
                    nc.tensor.matmul(pc[:], lhsT=w1c_b[:, fsl], rhs=xcTb[:, gs], start=False, stop=False)
                    nc.tensor.matmul(pc[:], lhsT=cU9[:, fsl], rhs=H9c[:, gs], start=False, stop=True)
                    nc.vector.tensor_scalar(
                        out=c1s[:, fb * GRP : (fb + 1) * GRP], in0=pc[:],
                        scalar1=b1c[:, fb : fb + 1], scalar2=None, op0=ALU.add,
                    )
                # L2 + relu: h2T [512, GRP] in 4 blocks
                h2s = mlp.tile([128, 4 * GRP], f32, tag="h2s")
                c2s = mlp.tile([128, 4 * GRP], f32, tag="c2s")
                for fb in range(4):
                    fsl = slice(fb * 128, (fb + 1) * 128)
                    p2 = psL.tile([128, GRP], f32, tag="pl2")
                    nc.tensor.matmul(p2[:], lhsT=w2h[:, fsl], rhs=h1s[:, 0:GRP], start=True, stop=False)
                    nc.tensor.matmul(p2[:], lhsT=w2h[:, 512 + fb * 128 : 512 + (fb + 1) * 128], rhs=h1s[:, GRP : 2 * GRP], start=False, stop=True)
                    nc.scalar.activation(
                        h2s[:, fb * GRP : (fb + 1) * GRP], p2[:],
                        mybir.ActivationFunctionType.Relu, bias=b2h[:, fb : fb + 1],
                    )
                    p2c = psL.tile([128, GRP], f32, tag="pl2")
                    nc.tensor.matmul(p2c[:], lhsT=w2c[:, fsl], rhs=c1s[:, 0:GRP], start=True, stop=False)
                    nc.tensor.matmul(p2c[:], lhsT=w2c[:, 512 + fb * 128 : 512 + (fb + 1) * 128], rhs=c1s[:, GRP : 2 * GRP], start=False, stop=True)
                    nc.scalar.activation(
                        c2s[:, fb * GRP : (fb + 1) * GRP], p2c[:],
                        mybir.ActivationFunctionType.Relu, bias=b2c[:, fb : fb + 1],
                    )
                # L3: out rows [2, GRP]  (row0 = c, row1 = h)
                p3h = psS.tile([1, GRP], f32, tag="p9")
                for kc in range(4):
                    nc.tensor.matmul(
                        p3h[:], lhsT=w3h[:, kc : kc + 1],
                        rhs=h2s[:, kc * GRP : (kc + 1) * GRP],
                        start=(kc == 0), stop=(kc == 3),
                    )
                nc.vector.tensor_scalar(
                    out=o2h[:, gs], in0=p3h[:], scalar1=b3h[:], scalar2=None, op0=ALU.add
                )
                p3c = psS.tile([1, GRP], f32, tag="p9")
                for kc in range(4):
                    nc.tensor.matmul(
                        p3c[:], lhsT=w3c[:, kc : kc + 1],
                        rhs=c2s[:, kc * GRP : (kc + 1) * GRP],
                        start=(kc == 0), stop=(kc == 3),
                    )
                nc.vector.tensor_scalar(
                    out=o2c[:, gs], in0=p3c[:], scalar1=b3c[:], scalar2=None, op0=ALU.add
                )
            nc.sync.dma_start(out=out[0:1, :], in_=o2c[:])
            nc.sync.dma_start(out=out[1:2, :], in_=o2h[:])
    _split_waits(nc)
    return nc


_NC_CACHE = {}


def _get_nc():
    if "nc" not in _NC_CACHE:
        _NC_CACHE["nc"] = _build()
    return _NC_CACHE["nc"]


# ---------------------------------------------------------------------------
# host side
# ---------------------------------------------------------------------------
def _pack_core(src_l, dst, sol_e, deg_inv_map, order_nodes):
    """Pack this core's valid edges (sorted by src) into NCOL static columns:
    column i holds edges of carbon output-slots [i*RPC, (i+1)*RPC), <=128 edges.
    Returns per-core device arrays + slot->node mapping."""
    vdst = np.zeros((128, NCOL), np.int32)
    vsol = np.zeros((128, NCOL), np.int32)
    vloc = np.zeros((128, NCOL), np.int32)
    vw = np.zeros((128, NCOL), np.float32)
    cxid = np.zeros(SLOTS, np.int32)
    csol = np.zeros(SLOTS, np.int32)
    inv = np.ones(SLOTS, np.float32)
    slot_node = np.full(SLOTS, -1, np.int64)

    # greedy pack: iterate has_h carbons in node order
    col = 0
    col_edges = 0
    col_ranks = 0
    eptr = 0
    ne = len(src_l)
    for node in order_nodes:
        d = deg_inv_map[node]
        if col_ranks >= RPC or col_edges + d > 128:
            col += 1
            col_edges = 0
            col_ranks = 0
        assert col < NCOL, "column capacity exceeded"
        slot = col * RPC + col_ranks
        slot_node[slot] = node
        inv[slot] = 1.0 / d
        for _ in range(d):
            e = eptr
            eptr += 1
            p = col_edges
            vdst[p, col] = dst[e]
            vsol[p, col] = sol_e[e]
            vloc[p, col] = col_ranks
            vw[p, col] = 1.0
            col_edges += 1
        col_ranks += 1
    assert eptr == ne
    return vdst, vsol, vloc, vw, cxid, csol, inv, slot_node


def prepare_in_maps(x, z, batch, edge_index, solvent_class,
                    c_emb, h_emb,
                    cW1, cb1, cW2, cb2, cW3, cb3,
                    hW1, hb1, hW2, hb2, hW3, hb3):
    maps, metas = _prepare(x, z, batch, edge_index, solvent_class,
                           c_emb, h_emb, cW1, cb1, cW2, cb2, cW3, cb3,
                           hW1, hb1, hW2, hb2, hW3, hb3)
    return maps


def _prepare(x, z, batch, edge_index, solvent_class,
             c_emb, h_emb,
             cW1, cb1, cW2, cb2, cW3, cb3,
             hW1, hb1, hW2, hb2, hW3, hb3):
    x = np.ascontiguousarray(np.asarray(x, np.float32))
    z = np.asarray(z).reshape(-1).astype(np.int64)
    batch = np.asarray(batch).reshape(-1).astype(np.int64)
    edge_index = np.asarray(edge_index).astype(np.int64)
    solvent_class = np.asarray(solvent_class).reshape(-1).astype(np.int64)

    n = x.shape[0]
    src, dst = edge_index[0], edge_index[1]
    is_c = z == 5
    is_h = z == 0
    valid = is_c[src] & is_h[dst]
    vs, vd = src[valid], dst[valid]
    sol_node = solvent_class[batch]

    # order valid edges by (core, src)
    order = np.lexsort((vd, vs))
    vs, vd = vs[order], vd[order]
    sol_e = sol_node[vd].astype(np.int32)

    deg = np.bincount(vs, minlength=n)

    in_maps = []
    metas = []
    shared = {
        "x": x,
        "c_emb": np.asarray(c_emb, np.float32), "h_emb": np.asarray(h_emb, np.float32),
        "cW1": np.asarray(cW1, np.float32), "cb1": np.asarray(cb1, np.float32),
        "cW2": np.asarray(cW2, np.float32), "cb2": np.asarray(cb2, np.float32),
        "cW3": np.asarray(cW3, np.float32), "cb3": np.asarray(cb3, np.float32),
        "hW1": np.asarray(hW1, np.float32), "hb1": np.asarray(hb1, np.float32),
        "hW2": np.asarray(hW2, np.float32), "hb2": np.asarray(hb2, np.float32),
        "hW3": np.asarray(hW3, np.float32), "hb3": np.asarray(hb3, np.float32),
    }
    core_of = vs // CH
    for c in range(NCORES):
        m = core_of == c
        cs, cd, csl = vs[m], vd[m], sol_e[m]
        nodes = np.unique(cs)  # sorted has_h carbons of this core
        vdst_a, vsol_a, vloc_a, vw_a, cxid_a, csol_a, inv_a, slot_node = _pack_core(
            cs, cd, csl, deg, nodes
        )
        used = slot_node >= 0
        cxid_a[used] = slot_node[used]
        csol_a[used] = sol_node[slot_node[used]]
        # column-major [128, CCOL] layout for gathers: slot = u*128 + p
        cxid_t = cxid_a.reshape(SLOTS // 128, 128).T.copy()
        csol_t = csol_a.reshape(SLOTS // 128, 128).T.copy()
        invrep = np.broadcast_to(inv_a, (128, SLOTS)).copy()
        in_map = dict(shared)
        in_map.update(
            vdst=vdst_a, vsol=vsol_a, vloc=vloc_a, vw=vw_a,
            cxid=cxid_t, csol=csol_t, invr=invrep,
        )
        in_maps.append(in_map)
        metas.append(slot_node)
    return in_maps, metas


def kernel(**inputs):
    in_maps, metas = _prepare(**inputs)
    nc = _get_nc()
    res = bass_utils.run_bass_kernel_spmd(nc, in_maps, core_ids=list(range(NCORES)))
    n = inputs["x"].shape[0]
    out_full = np.zeros((n, 2), np.float32)
    for c in range(NCORES):
        o2 = res.results[c]["out"]  # [2, SLOTS] rows: 0=c, 1=h
        slot_node = metas[c]
        used = slot_node >= 0
        nodes = slot_node[used]
        # device slot s maps rank at column-major order? o2 columns are slot ids
        out_full[nodes, 0] = o2[0, used]
        out_full[nodes, 1] = o2[1, used]
    return out_full
